# revision 27
# baseline (speedup 1.0000x reference)
"""Trainium2 Bass kernel for NodeReadout: out = relu(concat([node_feature, segment_sum(edge_state, edge_dst)]) @ W + b).

Strategy (8 NeuronCores, no collectives):
  - Shard edges by DESTINATION OWNER: core c owns nodes [c*12500, (c+1)*12500)
    and receives exactly the edges destined to its nodes.
  - Host-side sharding lays each core's edge_state out in padded-CSR order
    (edges grouped by destination node, nodes grouped by padded degree,
    features transposed so SBUF partitions = feature dims). Each node's edge
    list is split into two halves mapped to partition ranges [0:64) / [64:128)
    so the DVE segment-reduction uses all 128 lanes.
  - All streams are fp16 (tolerance is 2e-2; measured pipeline err ~4e-4),
    halving HBM traffic and enabling DVE 2x packed modes.
  - Device: per degree-group pairwise add-tree on the DVE (tensor_add at
    2x; tensor_reduce only has a 1x uop) folds each node's run to TWO
    partials stored interleaved; a 4-matmul PSUM accumulation (W1.T@nf +
    W22.T@pairA + W22.T@pairB) plus fused bias+ReLU (scalar engine)
    produces the output.
  - All 8 cores run the same NEFF with identical shapes (group structure is
    the per-degree max across cores; shortfall padded with zero rows / dummy
    node slots whose outputs are discarded on unshard).
"""

import math
import os
import sys
import types

import numpy as np

for _p in (
    "/root/.axon_site",
    "/root/.axon_site/_ro/trn_rl_repo",
    "/opt/trn_rl_repo",
):
    if os.path.isdir(_p) and _p not in sys.path:
        sys.path.append(_p)

N_CORES = 8
D = 64
SLAB = 512  # dense slab width (one PSUM bank of fp32)
CHUNK_ELEMS = int(os.environ.get("GNN_CHUNK", "4096"))
EBUF_BUFS = int(os.environ.get("GNN_EBUFS", "8"))


def _chunk_plan(groups, NSLOT, E_main):
    """Pack the contiguous MAIN edge_t stream into uniform DMA chunks.
    groups: (h_eff, ng, s_off, e_off, fold_src) where fold_src >= 0 marks a
    folded group whose B half-stream (same length as the main A part) lives
    at that edge_t offset and is CCE-accumulated onto the A span in SBUF.
    Each chunk = (elem_off, n_elems, segs, folds); seg = (local_elem_off,
    h_eff, n_nodes, slab_idx, slab_local_col); fold = (local_dest_off,
    src_off, n_elems). Chunk boundaries always fall on node boundaries."""
    segs = []  # (abs_eoA, h_eff, n_nodes, abs_col, fold_src_abs)
    for h, ng, s_off, e_off, fsrc in groups:
        s = 0
        while s < ng:
            col = s_off + s
            cn = min(ng - s, SLAB - col % SLAB)
            segs.append(
                (e_off + s * h, h, cn, col, -1 if fsrc < 0 else fsrc + s * h)
            )
            s += cn
    plan = []
    cur_eo, cur_fe, cur_segs, cur_folds = None, 0, [], []

    def flush():
        nonlocal cur_eo, cur_fe, cur_segs, cur_folds
        merged = []
        for f in cur_folds:
            if merged and merged[-1][0] + merged[-1][2] == f[0] and (
                merged[-1][1] + merged[-1][2] == f[1]
            ):
                merged[-1] = (merged[-1][0], merged[-1][1], merged[-1][2] + f[2])
            else:
                merged.append(f)
        plan.append((cur_eo, cur_fe, cur_segs, merged))
        cur_eo, cur_fe, cur_segs, cur_folds = None, 0, [], []

    for eo, h, cn, col, fsrc in segs:
        s = 0
        while s < cn:
            if cur_eo is None:
                cur_eo, cur_fe, cur_segs, cur_folds = eo + s * h, 0, [], []
            take = min(cn - s, (CHUNK_ELEMS - cur_fe) // h)
            if take == 0:
                flush()
                continue
            cur_segs.append(
                (cur_fe, h, take, (col + s) // SLAB, (col + s) % SLAB)
            )
            if fsrc >= 0:
                cur_folds.append((cur_fe, fsrc + s * h, take * h))
            cur_fe += take * h
            s += take
            if cur_fe > CHUNK_ELEMS - 1:
                flush()
    if cur_segs:
        flush()
    assert sum(fe for _, fe, _, _ in plan) == E_main
    return plan

_last_exec_time_ns = None
_last_results = None


def _install_shims():
    """Environment fixes: antenv.axon_hooks shim (NTFF profiling), no-op
    artifact upload, and a TileContext drain patch (this container's walrus
    rejects >1 sync-wait per instruction)."""
    # -- antenv.axon_hooks shim ------------------------------------------
    try:
        import antenv.axon_hooks  # noqa: F401
    except ImportError:
        try:
            import antenv

            mod = types.ModuleType("antenv.axon_hooks")
            mod._hook = None

            def set_axon_ntff_profile_hook(h):
                mod._hook = h

            def get_axon_ntff_profile_hook():
                return mod._hook

            mod.set_axon_ntff_profile_hook = set_axon_ntff_profile_hook
            mod.get_axon_ntff_profile_hook = get_axon_ntff_profile_hook
            sys.modules["antenv.axon_hooks"] = mod
            antenv.axon_hooks = mod
            try:
                from trn_agent_boot.trn_boot import _ntff_profile_via_ctypes

                so = "/opt/axon/libaxon_pjrt.so"
                if os.path.exists(so):
                    set_axon_ntff_profile_hook(_ntff_profile_via_ctypes(so))
            except Exception:
                pass
        except Exception:
            pass
    # -- artifact upload (needs a cloud bucket; not available here) ------
    try:
        import concourse.bass_utils as bu

        bu.upload_artifacts = lambda tmpdir: "local://" + tmpdir
    except Exception:
        pass
    # -- TileContext drain: split multi-sem waits ------------------------
    import concourse.mybir as mybir
    import concourse.tile as tile_mod
    from concourse.vector_clock import ScopedClock

    if getattr(tile_mod.TileContext, "_drain_patched", False):
        return
    tile_mod.TileContext._orig_drain_and_barrier = (
        tile_mod.TileContext._drain_and_barrier
    )

    def _drain_and_barrier(self, tick_clock, wait_clock):
        nc = self.nc
        probe = nc.sync.nop(nofuse=True, hint="drain_wait_split")
        wait_clock.add_sem_waits(
            probe.ins, ScopedClock({None: tick_clock.global_clock})
        )
        waits = list(probe.ins.sync_info.on_wait)
        probe.ins.sync_info.on_wait = waits[:1]
        for w in waits[1:]:
            nop = nc.sync.nop(nofuse=True, hint="drain_wait_split")
            nop.ins.sync_info = mybir.SyncInfo(on_update=[], on_wait=[w])
        nc.sync.drain()
        nc.all_engine_barrier()
        assert self.sems is not None
        popped = nc._tile_sem_poison_stack.pop()
        assert popped is self._sem_poison
        nc.clear_and_free_semaphores(list(self.sems.allocated().values()))
        nc.all_engine_barrier()

    tile_mod.TileContext._drain_and_barrier = _drain_and_barrier
    tile_mod.TileContext._patched_drain_and_barrier = _drain_and_barrier
    tile_mod.TileContext._drain_patched = True


def _split_multiwaits(nc):
    """Walrus here allows at most ONE sync-wait per instruction: hoist extra
    waits onto preceding NoOps on the same engine."""
    import concourse.mybir as mybir

    for fn in nc.m.functions:
        for blk in fn.blocks:
            insts = blk.instructions
            new = []
            for ins in insts:
                si = getattr(ins, "sync_info", None)
                waits = list(si.on_wait) if si is not None and si.on_wait else []
                if len(waits) > 1:
                    for j, w in enumerate(waits[:-1]):
                        nop = mybir.InstNoOp(
                            name=f"{ins.name}-wsplit{j}",
                            engine=ins.engine,
                            bass_nofuse=True,
                            sync_info=mybir.SyncInfo(on_update=[], on_wait=[w]),
                        )
                        new.append(nop)
                    si.on_wait = [waits[-1]]
                new.append(ins)
            blk.instructions[:] = new


def _prepare(node_feature, edge_state, edge_dst, W, b):
    """Host-side shard + layout. Returns (in_maps, groups, NSLOT, E2, col_node)."""
    node_feature = np.ascontiguousarray(np.asarray(node_feature), dtype=np.float32)
    edge_state16 = np.ascontiguousarray(np.asarray(edge_state), dtype=np.float16)
    edge_dst = np.asarray(edge_dst).astype(np.int64)
    W = np.ascontiguousarray(np.asarray(W), dtype=np.float16)
    b = np.asarray(b, dtype=np.float32).reshape(D, 1)

    N = node_feature.shape[0]
    # Global CSR: edges grouped by destination node.
    eid_sorted = np.argsort(edge_dst, kind="stable")
    deg = np.bincount(edge_dst, minlength=N)
    starts = np.cumsum(deg) - deg
    # Pad degree to a multiple of 4: per-half run length h = d/2 stays even
    # through every fold of the DVE add-tree, keeping all operands stride-1
    # and 4B-aligned (2x packed mode). ~6% extra edge bytes.
    degp = np.maximum(4, ((deg + 3) // 4) * 4)

    # Degree-balanced sharding: nodes sorted by padded degree are dealt
    # round-robin to cores, so per-core degree histograms match to within 1
    # and the common group structure carries almost no cross-core padding.
    rank = np.argsort(degp, kind="stable")  # node ids in degree order
    # per-core node lists, in degree order
    core_nodes = [rank[c::N_CORES] for c in range(N_CORES)]

    # Foldable groups (h % 4 == 0): each node's h-run is split into two
    # equal half-streams A|B; the B stream is CCE-accumulated onto A's SBUF
    # span by the DMA, so the DVE tree starts from h/2. Order foldable
    # groups first so the B ("fold") region stays contiguous per chunk.
    all_degs = sorted(int(v) for v in np.unique(degp))
    counts = {d: int(np.count_nonzero(degp == d)) for d in all_degs}
    fold_env = os.environ.get("GNN_FOLD", "1")
    if fold_env == "1":
        fold_degs = {d for d in all_degs if (d // 2) % 4 == 0}
    elif fold_env == "0":
        fold_degs = set()
    else:
        fold_degs = {int(x) for x in fold_env.split(",") if x}
    ordered = sorted(all_degs, key=lambda d: (d not in fold_degs, d))
    raw = []  # (d, n, s_off, e_off_main, h_eff, fold)
    s_off = 0
    e_off = 0
    for d in ordered:
        n = (counts[d] + N_CORES - 1) // N_CORES
        h = d // 2
        fold = d in fold_degs
        h_eff = h // 2 if fold else h
        raw.append((d, n, s_off, e_off, h_eff, fold))
        s_off += n
        e_off += n * h_eff
    NSLOT = s_off
    E_main = e_off
    fold_src = {}
    e_fold = E_main
    for d, n, so, eo, h_eff, fold in raw:
        if fold:
            fold_src[d] = e_fold
            e_fold += n * h_eff
    E2 = e_fold
    groups = [
        (h_eff, n, so, eo, fold_src.get(d, -1))
        for d, n, so, eo, h_eff, fold in raw
    ]

    in_maps = []
    col_node = np.full((N_CORES, NSLOT), -1, dtype=np.int64)
    for c in range(N_CORES):
        nodes = core_nodes[c]  # global ids, ascending degp
        ndeg = degp[nodes]
        gidx = np.full((2, E2), -1, dtype=np.int64)
        for d, n, so, eo, h_eff, fold in raw:
            nodes_d = nodes[ndeg == d]
            k = len(nodes_d)
            if k == 0:
                continue
            h = d // 2
            col = starts[nodes_d][:, None] + np.arange(d)[None, :]
            valid = np.arange(d)[None, :] < deg[nodes_d][:, None]
            em = np.where(valid, eid_sorted[np.where(valid, col, 0)], -1)
            em = em.reshape(k, 2, h)
            for half in range(2):
                if fold:
                    fo = fold_src[d]
                    gidx[half, eo : eo + k * h_eff] = em[
                        :, half, :h_eff
                    ].ravel()
                    gidx[half, fo : fo + k * h_eff] = em[
                        :, half, h_eff:
                    ].ravel()
                else:
                    gidx[half, eo : eo + k * h] = em[:, half, :].ravel()
            col_node[c, so : so + k] = nodes_d
        X = np.zeros((2, E2, D), dtype=np.float16)
        for half in range(2):
            m = gidx[half] >= 0
            X[half, m] = edge_state16[gidx[half, m]]
        edge_t = np.ascontiguousarray(
            X.transpose(0, 2, 1).reshape(2 * D, E2)
        )  # partitions [0:64)=half0 feats, [64:128)=half1 feats
        nf_t = np.zeros((D, NSLOT), dtype=np.float16)
        vm = col_node[c] >= 0
        nf_t[:, vm] = node_feature[col_node[c][vm]].T
        in_maps.append(
            {"edge_t": edge_t, "nf_t": nf_t, "W": W, "b": b}
        )
    return in_maps, groups, NSLOT, E_main, E2, col_node, N


def _build(groups, NSLOT, E_main, E2, for_sim=False):
    import concourse.bass as bass
    import concourse.mybir as mybir
    import concourse.tile as tile_mod
    from concourse.tile import TileContext

    if for_sim:
        # CoreSim can't digest the walrus single-wait workarounds; build
        # with the stock drain and skip the multi-wait split.
        tile_mod.TileContext._drain_and_barrier = (
            tile_mod.TileContext._orig_drain_and_barrier
        )

    f32 = mybir.dt.float32
    f16 = mybir.dt.float16
    nc = bass.Bass("TRN2", target_bir_lowering=False, debug=False)
    edge_t = nc.declare_dram_parameter("edge_t", [128, E2], f16, isOutput=False)
    nf_t = nc.declare_dram_parameter("nf_t", [64, NSLOT], f16, isOutput=False)
    Wp = nc.declare_dram_parameter("W", [128, D], f16, isOutput=False)
    bp = nc.declare_dram_parameter("b", [64, 1], f32, isOutput=False)
    out_t = nc.declare_dram_parameter("out_t", [64, NSLOT], f16, isOutput=True)

    with TileContext(nc) as tc, nc.allow_low_precision(
        reason="fp16 streams: tolerance is 2e-2; fp16 segment-sum err ~1e-3"
    ):
        with (
            tc.tile_pool(name="const", bufs=1) as cpool,
            tc.tile_pool(name="big", bufs=1) as bigpool,
            tc.tile_pool(name="edges", bufs=EBUF_BUFS) as epool,
            tc.tile_pool(name="scratch", bufs=3) as spool,
            tc.tile_pool(name="psum", bufs=4, space="PSUM") as ppool,
            tc.tile_pool(name="outs", bufs=3) as opool,
        ):
            # Matmul operands must sit at base partition 0 on this HW, so:
            # m1: lhsT=W1 [64,64], rhs=nf [64,:]; m2: lhsT=[W2;W2] [128,64],
            # rhs=agg [128,:] (sums both halves in one K=128 matmul).
            w1 = cpool.tile([64, D], f16)
            nc.scalar.dma_start(out=w1[:], in_=Wp[0:64, :])
            w22 = cpool.tile([128, D], f16)
            nc.scalar.dma_start(out=w22[0:64, :], in_=Wp[64:128, :])
            nc.scalar.dma_start(out=w22[64:128, :], in_=Wp[64:128, :])
            bt = cpool.tile([64, 1], f32)
            nc.scalar.dma_start(out=bt[:], in_=bp[:])

            # Per-slab agg tiles hold an interleaved PAIR of partial sums per
            # node slot (cols 2c/2c+1): the DVE add-tree stops at 2 partials
            # and the PE absorbs the last reduction via two accumulating
            # K=128 matmuls (stride-2 rhs columns). A dense slab depends only
            # on the tree ops that wrote its own tile, so matmul/ACT/out-DMA
            # interleave with the aggregation stream.
            n_slab = (NSLOT + SLAB - 1) // SLAB
            aggs = [
                bigpool.tile([128, 2 * SLAB], f16, name=f"agg{i}", tag=f"agg{i}")
                for i in range(n_slab)
            ]
            def dense_slab(sl):
                s = sl * SLAB
                n = min(SLAB, NSLOT - s)
                nfs = opool.tile([64, SLAB], f16, tag="nfs", name=f"nfs{sl}")
                nc.gpsimd.dma_start(out=nfs[:, :n], in_=nf_t[:, s : s + n])
                ps = ppool.tile(
                    [64, SLAB], f32, space="PSUM", tag="ps", name=f"ps{sl}"
                )
                nc.tensor.matmul(
                    out=ps[:, :n],
                    lhsT=w1[:],
                    rhs=nfs[:, :n],
                    start=True,
                    stop=False,
                )
                pairs = aggs[sl][:, : 2 * n].rearrange(
                    "p (n two) -> p n two", two=2
                )
                nc.tensor.matmul(
                    out=ps[:, :n],
                    lhsT=w22[:],
                    rhs=pairs[:, :, 0],
                    start=False,
                    stop=False,
                )
                nc.tensor.matmul(
                    out=ps[:, :n],
                    lhsT=w22[:],
                    rhs=pairs[:, :, 1],
                    start=False,
                    stop=True,
                )
                ob = opool.tile([64, SLAB], f16, tag="ob", name=f"ob{sl}")
                nc.scalar.activation(
                    out=ob[:, :n],
                    in_=ps[:, :n],
                    func=mybir.ActivationFunctionType.Relu,
                    bias=bt[:],
                )
                nc.gpsimd.dma_start(out=out_t[:, s : s + n], in_=ob[:, :n])

            # Uniform-size DMA chunks over the contiguous edge stream; the
            # per-group/per-slab reduce segments read from within the chunk.
            # Dense work for a slab is emitted right after the chunk that
            # completes it, so PE/ACT/out-DMA trail the stream closely.
            plan = _chunk_plan(groups, NSLOT, E_main)
            last_chunk_of_slab = {}
            for ci, (_, _, segs, _) in enumerate(plan):
                for _, _, _, sl, _ in segs:
                    last_chunk_of_slab[sl] = ci
            edge_qs = [nc.sync, nc.scalar]
            def V(buf, off, stride, cn, k0, k1):
                # [p][cn nodes @ stride][k0:k1] packed-run view
                return buf[:, off : off + cn * stride].rearrange(
                    "p (n k) -> p n k", k=stride
                )[:, :, k0:k1]

            def emit_seg(ebuf, scratch, cur, loff, h, cn, sl, lc):
                # Pairwise fold h (even) down to a 2-partial interleaved pair
                # in aggs[sl][:, 2lc:2lc+2cn]. All adds/copies keep stride-1
                # even-length inner runs at even offsets -> DVE 2x/4x modes.
                buf, off, st, hh = ebuf, loff, h, h
                while hh > 4:
                    k = 2 * (hh // 4)
                    rem = hh - 2 * k  # 0 or 2 (hh even)
                    nh = k + rem
                    nc.vector.tensor_add(
                        V(scratch, cur, nh, cn, 0, k),
                        V(buf, off, st, cn, 0, k),
                        V(buf, off, st, cn, k, 2 * k),
                    )
                    if rem:
                        nc.vector.tensor_copy(
                            V(scratch, cur, nh, cn, k, nh),
                            V(buf, off, st, cn, 2 * k, hh),
                        )
                    buf, off, st, hh = scratch, cur, nh, nh
                    cur += cn * nh
                o2 = aggs[sl][:, 2 * lc : 2 * lc + 2 * cn].rearrange(
                    "p (n two) -> p n two", two=2
                )
                if hh == 4:
                    nc.vector.tensor_add(
                        o2, V(buf, off, st, cn, 0, 2), V(buf, off, st, cn, 2, 4)
                    )
                else:  # hh == 2: pairs already contiguous, straight copy
                    nc.vector.tensor_copy(
                        aggs[sl][:, 2 * lc : 2 * lc + 2 * cn],
                        buf[:, off : off + 2 * cn],
                    )
                return cur

            for ci, (eo, fe, segs, folds) in enumerate(plan):
                ebuf = epool.tile([128, CHUNK_ELEMS], f16, tag="ebuf")
                dma_eng = edge_qs[ci % len(edge_qs)]
                dma_eng.dma_start(out=ebuf[:, :fe], in_=edge_t[:, eo : eo + fe])
                # B half-streams: the DMA's CCE unit adds them onto the A
                # span in SBUF (gpsimd/SWDGE is the only accum-capable path).
                # Accum DMAs above ~0.5MB crash the runtime: split to <=2048
                # elems (128 x 4KB rows) per instruction.
                for dloff, fsrc, flen in folds:
                    for o in range(0, flen, 2048):
                        ln = min(2048, flen - o)
                        nc.gpsimd.dma_start(
                            out=ebuf[:, dloff + o : dloff + o + ln],
                            in_=edge_t[:, fsrc + o : fsrc + o + ln],
                            accum_op=mybir.AluOpType.add,
                        )
                scratch = None
                if any(h > 4 for _, h, _, _, _ in segs):
                    scratch = spool.tile(
                        [128, CHUNK_ELEMS + CHUNK_ELEMS // 4],
                        f16,
                        tag="scr",
                        name=f"scr{ci}",
                    )
                cur = 0
                for loff, h, cn, sl, lc in segs:
                    cur = emit_seg(ebuf, scratch, cur, loff, h, cn, sl, lc)
                for sl in sorted(
                    s for s, lc in last_chunk_of_slab.items() if lc == ci
                ):
                    dense_slab(sl)
    if for_sim:
        # restore the patched drain for subsequent HW builds
        tile_mod.TileContext._drain_and_barrier = (
            tile_mod.TileContext._patched_drain_and_barrier
        )
    else:
        _split_multiwaits(nc)
    return nc


def kernel(node_feature, edge_state, edge_dst, W, b):
    global _last_exec_time_ns, _last_results
    _install_shims()
    from concourse.bass_utils import run_bass_kernel_spmd

    in_maps, groups, NSLOT, E_main, E2, col_node, N = _prepare(
        node_feature, edge_state, edge_dst, W, b
    )
    nc = _build(groups, NSLOT, E_main, E2)
    trace = bool(os.environ.get("GNN_TRACE"))
    res = run_bass_kernel_spmd(
        nc, in_maps, core_ids=list(range(N_CORES)), trace=trace
    )
    _last_exec_time_ns = res.exec_time_ns
    _last_results = res
    out = np.zeros((N, D), dtype=np.float32)
    for c in range(N_CORES):
        ot = np.asarray(res.results[c]["out_t"]).astype(np.float32)
        vm = col_node[c] >= 0
        out[col_node[c][vm]] = ot[:, vm].T
    return out


def last_exec_time_ns():
    return _last_exec_time_ns


def last_results():
    return _last_results



# revision 29
# speedup vs baseline: 1.0026x; 1.0026x over previous
"""Trainium2 Bass kernel for NodeReadout: out = relu(concat([node_feature, segment_sum(edge_state, edge_dst)]) @ W + b).

Strategy (8 NeuronCores, no collectives):
  - Shard edges by DESTINATION OWNER: core c owns nodes [c*12500, (c+1)*12500)
    and receives exactly the edges destined to its nodes.
  - Host-side sharding lays each core's edge_state out in padded-CSR order
    (edges grouped by destination node, nodes grouped by padded degree,
    features transposed so SBUF partitions = feature dims). Each node's edge
    list is split into two halves mapped to partition ranges [0:64) / [64:128)
    so the DVE segment-reduction uses all 128 lanes.
  - All streams are fp16 (tolerance is 2e-2; measured pipeline err ~4e-4),
    halving HBM traffic and enabling DVE 2x packed modes.
  - Device: per degree-group pairwise add-tree on the DVE (tensor_add at
    2x; tensor_reduce only has a 1x uop) folds each node's run to TWO
    partials stored interleaved; a 4-matmul PSUM accumulation (W1.T@nf +
    W22.T@pairA + W22.T@pairB) plus fused bias+ReLU (scalar engine)
    produces the output.
  - All 8 cores run the same NEFF with identical shapes (group structure is
    the per-degree max across cores; shortfall padded with zero rows / dummy
    node slots whose outputs are discarded on unshard).
"""

import math
import os
import sys
import types

import numpy as np

for _p in (
    "/root/.axon_site",
    "/root/.axon_site/_ro/trn_rl_repo",
    "/opt/trn_rl_repo",
):
    if os.path.isdir(_p) and _p not in sys.path:
        sys.path.append(_p)

N_CORES = 8
D = 64
SLAB = 512  # dense slab width (one PSUM bank of fp32)
CHUNK_ELEMS = int(os.environ.get("GNN_CHUNK", "4096"))
EBUF_BUFS = int(os.environ.get("GNN_EBUFS", "8"))


def _chunk_plan(groups, NSLOT, E_main):
    """Pack the contiguous MAIN edge_t stream into uniform DMA chunks.
    groups: (h_eff, ng, s_off, e_off, fold_src) where fold_src >= 0 marks a
    folded group whose B half-stream (same length as the main A part) lives
    at that edge_t offset and is CCE-accumulated onto the A span in SBUF.
    Each chunk = (elem_off, n_elems, segs, folds); seg = (local_elem_off,
    h_eff, n_nodes, slab_idx, slab_local_col); fold = (local_dest_off,
    src_off, n_elems). Chunk boundaries always fall on node boundaries."""
    segs = []  # (abs_eoA, h_eff, n_nodes, abs_col, fold_src_abs)
    for h, ng, s_off, e_off, fsrc in groups:
        s = 0
        while s < ng:
            col = s_off + s
            cn = min(ng - s, SLAB - col % SLAB)
            segs.append(
                (e_off + s * h, h, cn, col, -1 if fsrc < 0 else fsrc + s * h)
            )
            s += cn
    plan = []
    cur_eo, cur_fe, cur_segs, cur_folds = None, 0, [], []

    def flush():
        nonlocal cur_eo, cur_fe, cur_segs, cur_folds
        merged = []
        for f in cur_folds:
            if merged and merged[-1][0] + merged[-1][2] == f[0] and (
                merged[-1][1] + merged[-1][2] == f[1]
            ):
                merged[-1] = (merged[-1][0], merged[-1][1], merged[-1][2] + f[2])
            else:
                merged.append(f)
        plan.append((cur_eo, cur_fe, cur_segs, merged))
        cur_eo, cur_fe, cur_segs, cur_folds = None, 0, [], []

    for eo, h, cn, col, fsrc in segs:
        s = 0
        while s < cn:
            if cur_eo is None:
                cur_eo, cur_fe, cur_segs, cur_folds = eo + s * h, 0, [], []
            take = min(cn - s, (CHUNK_ELEMS - cur_fe) // h)
            if take == 0:
                flush()
                continue
            cur_segs.append(
                (cur_fe, h, take, (col + s) // SLAB, (col + s) % SLAB)
            )
            if fsrc >= 0:
                cur_folds.append((cur_fe, fsrc + s * h, take * h))
            cur_fe += take * h
            s += take
            if cur_fe > CHUNK_ELEMS - 1:
                flush()
    if cur_segs:
        flush()
    assert sum(fe for _, fe, _, _ in plan) == E_main
    return plan

_last_exec_time_ns = None
_last_results = None


def _install_shims():
    """Environment fixes: antenv.axon_hooks shim (NTFF profiling), no-op
    artifact upload, and a TileContext drain patch (this container's walrus
    rejects >1 sync-wait per instruction)."""
    # -- antenv.axon_hooks shim ------------------------------------------
    try:
        import antenv.axon_hooks  # noqa: F401
    except ImportError:
        try:
            import antenv

            mod = types.ModuleType("antenv.axon_hooks")
            mod._hook = None

            def set_axon_ntff_profile_hook(h):
                mod._hook = h

            def get_axon_ntff_profile_hook():
                return mod._hook

            mod.set_axon_ntff_profile_hook = set_axon_ntff_profile_hook
            mod.get_axon_ntff_profile_hook = get_axon_ntff_profile_hook
            sys.modules["antenv.axon_hooks"] = mod
            antenv.axon_hooks = mod
            try:
                from trn_agent_boot.trn_boot import _ntff_profile_via_ctypes

                so = "/opt/axon/libaxon_pjrt.so"
                if os.path.exists(so):
                    set_axon_ntff_profile_hook(_ntff_profile_via_ctypes(so))
            except Exception:
                pass
        except Exception:
            pass
    # -- artifact upload (needs a cloud bucket; not available here) ------
    try:
        import concourse.bass_utils as bu

        bu.upload_artifacts = lambda tmpdir: "local://" + tmpdir
    except Exception:
        pass
    # -- TileContext drain: split multi-sem waits ------------------------
    import concourse.mybir as mybir
    import concourse.tile as tile_mod
    from concourse.vector_clock import ScopedClock

    if getattr(tile_mod.TileContext, "_drain_patched", False):
        return
    tile_mod.TileContext._orig_drain_and_barrier = (
        tile_mod.TileContext._drain_and_barrier
    )

    def _drain_and_barrier(self, tick_clock, wait_clock):
        nc = self.nc
        probe = nc.sync.nop(nofuse=True, hint="drain_wait_split")
        wait_clock.add_sem_waits(
            probe.ins, ScopedClock({None: tick_clock.global_clock})
        )
        waits = list(probe.ins.sync_info.on_wait)
        probe.ins.sync_info.on_wait = waits[:1]
        for w in waits[1:]:
            nop = nc.sync.nop(nofuse=True, hint="drain_wait_split")
            nop.ins.sync_info = mybir.SyncInfo(on_update=[], on_wait=[w])
        nc.sync.drain()
        nc.all_engine_barrier()
        assert self.sems is not None
        popped = nc._tile_sem_poison_stack.pop()
        assert popped is self._sem_poison
        nc.clear_and_free_semaphores(list(self.sems.allocated().values()))
        nc.all_engine_barrier()

    tile_mod.TileContext._drain_and_barrier = _drain_and_barrier
    tile_mod.TileContext._patched_drain_and_barrier = _drain_and_barrier
    tile_mod.TileContext._drain_patched = True


def _split_multiwaits(nc):
    """Walrus here allows at most ONE sync-wait per instruction: hoist extra
    waits onto preceding NoOps on the same engine."""
    import concourse.mybir as mybir

    for fn in nc.m.functions:
        for blk in fn.blocks:
            insts = blk.instructions
            new = []
            for ins in insts:
                si = getattr(ins, "sync_info", None)
                waits = list(si.on_wait) if si is not None and si.on_wait else []
                if len(waits) > 1:
                    for j, w in enumerate(waits[:-1]):
                        nop = mybir.InstNoOp(
                            name=f"{ins.name}-wsplit{j}",
                            engine=ins.engine,
                            bass_nofuse=True,
                            sync_info=mybir.SyncInfo(on_update=[], on_wait=[w]),
                        )
                        new.append(nop)
                    si.on_wait = [waits[-1]]
                new.append(ins)
            blk.instructions[:] = new


def _prepare(node_feature, edge_state, edge_dst, W, b):
    """Host-side shard + layout. Returns (in_maps, groups, NSLOT, E2, col_node)."""
    node_feature = np.ascontiguousarray(np.asarray(node_feature), dtype=np.float32)
    edge_state16 = np.ascontiguousarray(np.asarray(edge_state), dtype=np.float16)
    edge_dst = np.asarray(edge_dst).astype(np.int64)
    W = np.ascontiguousarray(np.asarray(W), dtype=np.float16)
    b = np.asarray(b, dtype=np.float32).reshape(D, 1)

    N = node_feature.shape[0]
    # Global CSR: edges grouped by destination node.
    eid_sorted = np.argsort(edge_dst, kind="stable")
    deg = np.bincount(edge_dst, minlength=N)
    starts = np.cumsum(deg) - deg
    # Pad degree to a multiple of 4: per-half run length h = d/2 stays even
    # through every fold of the DVE add-tree, keeping all operands stride-1
    # and 4B-aligned (2x packed mode). ~6% extra edge bytes.
    degp = np.maximum(4, ((deg + 3) // 4) * 4)

    # Degree-balanced sharding: nodes sorted by padded degree are dealt
    # round-robin to cores, so per-core degree histograms match to within 1
    # and the common group structure carries almost no cross-core padding.
    rank = np.argsort(degp, kind="stable")  # node ids in degree order
    # per-core node lists, in degree order
    core_nodes = [rank[c::N_CORES] for c in range(N_CORES)]

    # Foldable groups (h % 4 == 0): each node's h-run is split into two
    # equal half-streams A|B; the B stream is CCE-accumulated onto A's SBUF
    # span by the DMA, so the DVE tree starts from h/2. Order foldable
    # groups first so the B ("fold") region stays contiguous per chunk.
    all_degs = sorted(int(v) for v in np.unique(degp))
    counts = {d: int(np.count_nonzero(degp == d)) for d in all_degs}
    fold_env = os.environ.get("GNN_FOLD", "1")
    if fold_env == "1":
        fold_degs = {d for d in all_degs if (d // 2) % 4 == 0}
    elif fold_env == "0":
        fold_degs = set()
    else:
        fold_degs = {int(x) for x in fold_env.split(",") if x}
    ordered = sorted(all_degs, key=lambda d: (d not in fold_degs, d))
    raw = []  # (d, n, s_off, e_off_main, h_eff, fold)
    s_off = 0
    e_off = 0
    for d in ordered:
        n = (counts[d] + N_CORES - 1) // N_CORES
        h = d // 2
        fold = d in fold_degs
        h_eff = h // 2 if fold else h
        raw.append((d, n, s_off, e_off, h_eff, fold))
        s_off += n
        e_off += n * h_eff
    NSLOT = s_off
    E_main = e_off
    fold_src = {}
    e_fold = E_main
    for d, n, so, eo, h_eff, fold in raw:
        if fold:
            fold_src[d] = e_fold
            e_fold += n * h_eff
    E2 = e_fold
    groups = [
        (h_eff, n, so, eo, fold_src.get(d, -1))
        for d, n, so, eo, h_eff, fold in raw
    ]

    in_maps = []
    col_node = np.full((N_CORES, NSLOT), -1, dtype=np.int64)
    for c in range(N_CORES):
        nodes = core_nodes[c]  # global ids, ascending degp
        ndeg = degp[nodes]
        gidx = np.full((2, E2), -1, dtype=np.int64)
        for d, n, so, eo, h_eff, fold in raw:
            nodes_d = nodes[ndeg == d]
            k = len(nodes_d)
            if k == 0:
                continue
            h = d // 2
            col = starts[nodes_d][:, None] + np.arange(d)[None, :]
            valid = np.arange(d)[None, :] < deg[nodes_d][:, None]
            em = np.where(valid, eid_sorted[np.where(valid, col, 0)], -1)
            em = em.reshape(k, 2, h)
            for half in range(2):
                if fold:
                    fo = fold_src[d]
                    gidx[half, eo : eo + k * h_eff] = em[
                        :, half, :h_eff
                    ].ravel()
                    gidx[half, fo : fo + k * h_eff] = em[
                        :, half, h_eff:
                    ].ravel()
                else:
                    gidx[half, eo : eo + k * h] = em[:, half, :].ravel()
            col_node[c, so : so + k] = nodes_d
        X = np.zeros((2, E2, D), dtype=np.float16)
        for half in range(2):
            m = gidx[half] >= 0
            X[half, m] = edge_state16[gidx[half, m]]
        edge_t = np.ascontiguousarray(
            X.transpose(0, 2, 1).reshape(2 * D, E2)
        )  # partitions [0:64)=half0 feats, [64:128)=half1 feats
        nf_t = np.zeros((D, NSLOT), dtype=np.float16)
        vm = col_node[c] >= 0
        nf_t[:, vm] = node_feature[col_node[c][vm]].T
        in_maps.append(
            {"edge_t": edge_t, "nf_t": nf_t, "W": W, "b": b}
        )
    return in_maps, groups, NSLOT, E_main, E2, col_node, N


def _build(groups, NSLOT, E_main, E2, for_sim=False):
    import concourse.bass as bass
    import concourse.mybir as mybir
    import concourse.tile as tile_mod
    from concourse.tile import TileContext

    if for_sim:
        # CoreSim can't digest the walrus single-wait workarounds; build
        # with the stock drain and skip the multi-wait split.
        tile_mod.TileContext._drain_and_barrier = (
            tile_mod.TileContext._orig_drain_and_barrier
        )

    f32 = mybir.dt.float32
    f16 = mybir.dt.float16
    nc = bass.Bass("TRN2", target_bir_lowering=False, debug=False)
    edge_t = nc.declare_dram_parameter("edge_t", [128, E2], f16, isOutput=False)
    nf_t = nc.declare_dram_parameter("nf_t", [64, NSLOT], f16, isOutput=False)
    Wp = nc.declare_dram_parameter("W", [128, D], f16, isOutput=False)
    bp = nc.declare_dram_parameter("b", [64, 1], f32, isOutput=False)
    out_t = nc.declare_dram_parameter("out_t", [64, NSLOT], f16, isOutput=True)

    with TileContext(nc) as tc, nc.allow_low_precision(
        reason="fp16 streams: tolerance is 2e-2; fp16 segment-sum err ~1e-3"
    ):
        with (
            tc.tile_pool(name="const", bufs=1) as cpool,
            tc.tile_pool(name="big", bufs=1) as bigpool,
            tc.tile_pool(name="edges", bufs=EBUF_BUFS) as epool,
            tc.tile_pool(name="scratch", bufs=3) as spool,
            tc.tile_pool(name="psum", bufs=4, space="PSUM") as ppool,
            tc.tile_pool(name="outs", bufs=3) as opool,
        ):
            # Matmul operands must sit at base partition 0 on this HW, so:
            # m1: lhsT=W1 [64,64], rhs=nf [64,:]; m2: lhsT=[W2;W2] [128,64],
            # rhs=agg [128,:] (sums both halves in one K=128 matmul).
            w1 = cpool.tile([64, D], f16)
            nc.scalar.dma_start(out=w1[:], in_=Wp[0:64, :])
            w22 = cpool.tile([128, D], f16)
            nc.scalar.dma_start(out=w22[0:64, :], in_=Wp[64:128, :])
            nc.scalar.dma_start(out=w22[64:128, :], in_=Wp[64:128, :])
            bt = cpool.tile([64, 1], f32)
            nc.scalar.dma_start(out=bt[:], in_=bp[:])

            # Per-slab agg tiles hold an interleaved PAIR of partial sums per
            # node slot (cols 2c/2c+1): the DVE add-tree stops at 2 partials
            # and the PE absorbs the last reduction via two accumulating
            # K=128 matmuls (stride-2 rhs columns). A dense slab depends only
            # on the tree ops that wrote its own tile, so matmul/ACT/out-DMA
            # interleave with the aggregation stream.
            n_slab = (NSLOT + SLAB - 1) // SLAB
            aggs = [
                bigpool.tile([128, 2 * SLAB], f16, name=f"agg{i}", tag=f"agg{i}")
                for i in range(n_slab)
            ]
            def dense_slab(sl):
                s = sl * SLAB
                n = min(SLAB, NSLOT - s)
                nfs = opool.tile([64, SLAB], f16, tag="nfs", name=f"nfs{sl}")
                nc.sync.dma_start(out=nfs[:, :n], in_=nf_t[:, s : s + n])
                ps = ppool.tile(
                    [64, SLAB], f32, space="PSUM", tag="ps", name=f"ps{sl}"
                )
                nc.tensor.matmul(
                    out=ps[:, :n],
                    lhsT=w1[:],
                    rhs=nfs[:, :n],
                    start=True,
                    stop=False,
                )
                pairs = aggs[sl][:, : 2 * n].rearrange(
                    "p (n two) -> p n two", two=2
                )
                nc.tensor.matmul(
                    out=ps[:, :n],
                    lhsT=w22[:],
                    rhs=pairs[:, :, 0],
                    start=False,
                    stop=False,
                )
                nc.tensor.matmul(
                    out=ps[:, :n],
                    lhsT=w22[:],
                    rhs=pairs[:, :, 1],
                    start=False,
                    stop=True,
                )
                ob = opool.tile([64, SLAB], f16, tag="ob", name=f"ob{sl}")
                nc.scalar.activation(
                    out=ob[:, :n],
                    in_=ps[:, :n],
                    func=mybir.ActivationFunctionType.Relu,
                    bias=bt[:],
                )
                nc.scalar.dma_start(out=out_t[:, s : s + n], in_=ob[:, :n])

            # Uniform-size DMA chunks over the contiguous edge stream; the
            # per-group/per-slab reduce segments read from within the chunk.
            # Dense work for a slab is emitted right after the chunk that
            # completes it, so PE/ACT/out-DMA trail the stream closely.
            plan = _chunk_plan(groups, NSLOT, E_main)
            last_chunk_of_slab = {}
            for ci, (_, _, segs, _) in enumerate(plan):
                for _, _, _, sl, _ in segs:
                    last_chunk_of_slab[sl] = ci
            edge_qs = [nc.sync, nc.scalar]
            def V(buf, off, stride, cn, k0, k1):
                # [p][cn nodes @ stride][k0:k1] packed-run view
                return buf[:, off : off + cn * stride].rearrange(
                    "p (n k) -> p n k", k=stride
                )[:, :, k0:k1]

            def emit_seg(ebuf, scratch, cur, loff, h, cn, sl, lc):
                # Pairwise fold h (even) down to a 2-partial interleaved pair
                # in aggs[sl][:, 2lc:2lc+2cn]. All adds/copies keep stride-1
                # even-length inner runs at even offsets -> DVE 2x/4x modes.
                buf, off, st, hh = ebuf, loff, h, h
                while hh > 4:
                    k = 2 * (hh // 4)
                    rem = hh - 2 * k  # 0 or 2 (hh even)
                    nh = k + rem
                    nc.vector.tensor_add(
                        V(scratch, cur, nh, cn, 0, k),
                        V(buf, off, st, cn, 0, k),
                        V(buf, off, st, cn, k, 2 * k),
                    )
                    if rem:
                        nc.vector.tensor_copy(
                            V(scratch, cur, nh, cn, k, nh),
                            V(buf, off, st, cn, 2 * k, hh),
                        )
                    buf, off, st, hh = scratch, cur, nh, nh
                    cur += cn * nh
                o2 = aggs[sl][:, 2 * lc : 2 * lc + 2 * cn].rearrange(
                    "p (n two) -> p n two", two=2
                )
                if hh == 4:
                    nc.vector.tensor_add(
                        o2, V(buf, off, st, cn, 0, 2), V(buf, off, st, cn, 2, 4)
                    )
                else:  # hh == 2: pairs already contiguous, straight copy
                    nc.vector.tensor_copy(
                        aggs[sl][:, 2 * lc : 2 * lc + 2 * cn],
                        buf[:, off : off + 2 * cn],
                    )
                return cur

            for ci, (eo, fe, segs, folds) in enumerate(plan):
                ebuf = epool.tile([128, CHUNK_ELEMS], f16, tag="ebuf")
                dma_eng = edge_qs[ci % len(edge_qs)]
                dma_eng.dma_start(out=ebuf[:, :fe], in_=edge_t[:, eo : eo + fe])
                # B half-streams: the DMA's CCE unit adds them onto the A
                # span in SBUF (gpsimd/SWDGE is the only accum-capable path).
                # Accum DMAs above ~0.5MB crash the runtime: split to <=2048
                # elems (128 x 4KB rows) per instruction.
                for dloff, fsrc, flen in folds:
                    for o in range(0, flen, 2048):
                        ln = min(2048, flen - o)
                        nc.gpsimd.dma_start(
                            out=ebuf[:, dloff + o : dloff + o + ln],
                            in_=edge_t[:, fsrc + o : fsrc + o + ln],
                            accum_op=mybir.AluOpType.add,
                        )
                scratch = None
                if any(h > 4 for _, h, _, _, _ in segs):
                    scratch = spool.tile(
                        [128, CHUNK_ELEMS + CHUNK_ELEMS // 4],
                        f16,
                        tag="scr",
                        name=f"scr{ci}",
                    )
                cur = 0
                for loff, h, cn, sl, lc in segs:
                    cur = emit_seg(ebuf, scratch, cur, loff, h, cn, sl, lc)
                for sl in sorted(
                    s for s, lc in last_chunk_of_slab.items() if lc == ci
                ):
                    dense_slab(sl)
    if for_sim:
        # restore the patched drain for subsequent HW builds
        tile_mod.TileContext._drain_and_barrier = (
            tile_mod.TileContext._patched_drain_and_barrier
        )
    else:
        _split_multiwaits(nc)
    return nc


def kernel(node_feature, edge_state, edge_dst, W, b):
    global _last_exec_time_ns, _last_results
    _install_shims()
    from concourse.bass_utils import run_bass_kernel_spmd

    in_maps, groups, NSLOT, E_main, E2, col_node, N = _prepare(
        node_feature, edge_state, edge_dst, W, b
    )
    nc = _build(groups, NSLOT, E_main, E2)
    trace = bool(os.environ.get("GNN_TRACE"))
    res = run_bass_kernel_spmd(
        nc, in_maps, core_ids=list(range(N_CORES)), trace=trace
    )
    _last_exec_time_ns = res.exec_time_ns
    _last_results = res
    out = np.zeros((N, D), dtype=np.float32)
    for c in range(N_CORES):
        ot = np.asarray(res.results[c]["out_t"]).astype(np.float32)
        vm = col_node[c] >= 0
        out[col_node[c][vm]] = ot[:, vm].T
    return out


def last_exec_time_ns():
    return _last_exec_time_ns


def last_results():
    return _last_results



# revision 30
# speedup vs baseline: 1.0906x; 1.0878x over previous
"""Trainium2 Bass kernel for NodeReadout: out = relu(concat([node_feature, segment_sum(edge_state, edge_dst)]) @ W + b).

Strategy (8 NeuronCores, no collectives):
  - Shard edges by DESTINATION OWNER: core c owns nodes [c*12500, (c+1)*12500)
    and receives exactly the edges destined to its nodes.
  - Host-side sharding lays each core's edge_state out in padded-CSR order
    (edges grouped by destination node, nodes grouped by padded degree,
    features transposed so SBUF partitions = feature dims). Each node's edge
    list is split into two halves mapped to partition ranges [0:64) / [64:128)
    so the DVE segment-reduction uses all 128 lanes.
  - All streams are fp16 (tolerance is 2e-2; measured pipeline err ~4e-4),
    halving HBM traffic and enabling DVE 2x packed modes.
  - Device: per degree-group pairwise add-tree on the DVE (tensor_add at
    2x; tensor_reduce only has a 1x uop) folds each node's run to TWO
    partials stored interleaved; a 4-matmul PSUM accumulation (W1.T@nf +
    W22.T@pairA + W22.T@pairB) plus fused bias+ReLU (scalar engine)
    produces the output.
  - All 8 cores run the same NEFF with identical shapes (group structure is
    the per-degree max across cores; shortfall padded with zero rows / dummy
    node slots whose outputs are discarded on unshard).
"""

import math
import os
import sys
import types

import numpy as np

for _p in (
    "/root/.axon_site",
    "/root/.axon_site/_ro/trn_rl_repo",
    "/opt/trn_rl_repo",
):
    if os.path.isdir(_p) and _p not in sys.path:
        sys.path.append(_p)

N_CORES = 8
D = 64
SLAB = 512  # dense slab width (one PSUM bank of fp32)
CHUNK_ELEMS = int(os.environ.get("GNN_CHUNK", "4096"))
EBUF_BUFS = int(os.environ.get("GNN_EBUFS", "8"))


def _chunk_plan(groups, NSLOT, E_main):
    """Pack the contiguous MAIN edge_t stream into uniform DMA chunks.
    groups: (h_eff, ng, s_off, e_off, fold_src) where fold_src >= 0 marks a
    folded group whose B half-stream (same length as the main A part) lives
    at that edge_t offset and is CCE-accumulated onto the A span in SBUF.
    Each chunk = (elem_off, n_elems, segs, folds); seg = (local_elem_off,
    h_eff, n_nodes, slab_idx, slab_local_col); fold = (local_dest_off,
    src_off, n_elems). Chunk boundaries always fall on node boundaries."""
    segs = []  # (abs_eoA, h_eff, n_nodes, abs_col, fold_src_abs)
    for h, ng, s_off, e_off, fsrc in groups:
        s = 0
        while s < ng:
            col = s_off + s
            cn = min(ng - s, SLAB - col % SLAB)
            segs.append(
                (e_off + s * h, h, cn, col, -1 if fsrc < 0 else fsrc + s * h)
            )
            s += cn
    plan = []
    cur_eo, cur_fe, cur_segs, cur_folds = None, 0, [], []

    def flush():
        nonlocal cur_eo, cur_fe, cur_segs, cur_folds
        merged = []
        for f in cur_folds:
            if merged and merged[-1][0] + merged[-1][2] == f[0] and (
                merged[-1][1] + merged[-1][2] == f[1]
            ):
                merged[-1] = (merged[-1][0], merged[-1][1], merged[-1][2] + f[2])
            else:
                merged.append(f)
        plan.append((cur_eo, cur_fe, cur_segs, merged))
        cur_eo, cur_fe, cur_segs, cur_folds = None, 0, [], []

    for eo, h, cn, col, fsrc in segs:
        s = 0
        while s < cn:
            if cur_eo is None:
                cur_eo, cur_fe, cur_segs, cur_folds = eo + s * h, 0, [], []
            take = min(cn - s, (CHUNK_ELEMS - cur_fe) // h)
            if take == 0:
                flush()
                continue
            cur_segs.append(
                (cur_fe, h, take, (col + s) // SLAB, (col + s) % SLAB)
            )
            if fsrc >= 0:
                cur_folds.append((cur_fe, fsrc + s * h, take * h))
            cur_fe += take * h
            s += take
            if cur_fe > CHUNK_ELEMS - 1:
                flush()
    if cur_segs:
        flush()
    assert sum(fe for _, fe, _, _ in plan) == E_main
    return plan

_last_exec_time_ns = None
_last_results = None


def _install_shims():
    """Environment fixes: antenv.axon_hooks shim (NTFF profiling), no-op
    artifact upload, and a TileContext drain patch (this container's walrus
    rejects >1 sync-wait per instruction)."""
    # -- antenv.axon_hooks shim ------------------------------------------
    try:
        import antenv.axon_hooks  # noqa: F401
    except ImportError:
        try:
            import antenv

            mod = types.ModuleType("antenv.axon_hooks")
            mod._hook = None

            def set_axon_ntff_profile_hook(h):
                mod._hook = h

            def get_axon_ntff_profile_hook():
                return mod._hook

            mod.set_axon_ntff_profile_hook = set_axon_ntff_profile_hook
            mod.get_axon_ntff_profile_hook = get_axon_ntff_profile_hook
            sys.modules["antenv.axon_hooks"] = mod
            antenv.axon_hooks = mod
            try:
                from trn_agent_boot.trn_boot import _ntff_profile_via_ctypes

                so = "/opt/axon/libaxon_pjrt.so"
                if os.path.exists(so):
                    set_axon_ntff_profile_hook(_ntff_profile_via_ctypes(so))
            except Exception:
                pass
        except Exception:
            pass
    # -- artifact upload (needs a cloud bucket; not available here) ------
    try:
        import concourse.bass_utils as bu

        bu.upload_artifacts = lambda tmpdir: "local://" + tmpdir
    except Exception:
        pass
    # -- TileContext drain: split multi-sem waits ------------------------
    import concourse.mybir as mybir
    import concourse.tile as tile_mod
    from concourse.vector_clock import ScopedClock

    if getattr(tile_mod.TileContext, "_drain_patched", False):
        return
    tile_mod.TileContext._orig_drain_and_barrier = (
        tile_mod.TileContext._drain_and_barrier
    )

    def _drain_and_barrier(self, tick_clock, wait_clock):
        nc = self.nc
        probe = nc.sync.nop(nofuse=True, hint="drain_wait_split")
        wait_clock.add_sem_waits(
            probe.ins, ScopedClock({None: tick_clock.global_clock})
        )
        waits = list(probe.ins.sync_info.on_wait)
        probe.ins.sync_info.on_wait = waits[:1]
        for w in waits[1:]:
            nop = nc.sync.nop(nofuse=True, hint="drain_wait_split")
            nop.ins.sync_info = mybir.SyncInfo(on_update=[], on_wait=[w])
        nc.sync.drain()
        nc.all_engine_barrier()
        assert self.sems is not None
        popped = nc._tile_sem_poison_stack.pop()
        assert popped is self._sem_poison
        nc.clear_and_free_semaphores(list(self.sems.allocated().values()))
        nc.all_engine_barrier()

    tile_mod.TileContext._drain_and_barrier = _drain_and_barrier
    tile_mod.TileContext._patched_drain_and_barrier = _drain_and_barrier
    tile_mod.TileContext._drain_patched = True


def _split_multiwaits(nc):
    """Walrus here allows at most ONE sync-wait per instruction: hoist extra
    waits onto preceding NoOps on the same engine."""
    import concourse.mybir as mybir

    for fn in nc.m.functions:
        for blk in fn.blocks:
            insts = blk.instructions
            new = []
            for ins in insts:
                si = getattr(ins, "sync_info", None)
                waits = list(si.on_wait) if si is not None and si.on_wait else []
                if len(waits) > 1:
                    for j, w in enumerate(waits[:-1]):
                        nop = mybir.InstNoOp(
                            name=f"{ins.name}-wsplit{j}",
                            engine=ins.engine,
                            bass_nofuse=True,
                            sync_info=mybir.SyncInfo(on_update=[], on_wait=[w]),
                        )
                        new.append(nop)
                    si.on_wait = [waits[-1]]
                new.append(ins)
            blk.instructions[:] = new


def _prepare(node_feature, edge_state, edge_dst, W, b):
    """Host-side shard + layout. Returns (in_maps, groups, NSLOT, E2, col_node)."""
    node_feature = np.ascontiguousarray(np.asarray(node_feature), dtype=np.float32)
    edge_state16 = np.ascontiguousarray(np.asarray(edge_state), dtype=np.float16)
    edge_dst = np.asarray(edge_dst).astype(np.int64)
    W = np.ascontiguousarray(np.asarray(W), dtype=np.float16)
    b = np.asarray(b, dtype=np.float32).reshape(D, 1)

    N = node_feature.shape[0]
    # Global CSR: edges grouped by destination node.
    eid_sorted = np.argsort(edge_dst, kind="stable")
    deg = np.bincount(edge_dst, minlength=N)
    starts = np.cumsum(deg) - deg
    # Pad degree to a multiple of 4: per-half run length h = d/2 stays even
    # through every fold of the DVE add-tree, keeping all operands stride-1
    # and 4B-aligned (2x packed mode). ~6% extra edge bytes.
    degp = np.maximum(4, ((deg + 3) // 4) * 4)

    # Degree-balanced sharding: nodes sorted by padded degree are dealt
    # round-robin to cores, so per-core degree histograms match to within 1
    # and the common group structure carries almost no cross-core padding.
    rank = np.argsort(degp, kind="stable")  # node ids in degree order
    # per-core node lists, in degree order
    core_nodes = [rank[c::N_CORES] for c in range(N_CORES)]

    # Foldable groups (h % 4 == 0): each node's h-run is split into two
    # equal half-streams A|B; the B stream is CCE-accumulated onto A's SBUF
    # span by the DMA, so the DVE tree starts from h/2. Order foldable
    # groups first so the B ("fold") region stays contiguous per chunk.
    all_degs = sorted(int(v) for v in np.unique(degp))
    counts = {d: int(np.count_nonzero(degp == d)) for d in all_degs}
    fold_env = os.environ.get("GNN_FOLD", "1")
    if fold_env == "1":
        fold_degs = {d for d in all_degs if (d // 2) % 4 == 0}
    elif fold_env == "0":
        fold_degs = set()
    else:
        fold_degs = {int(x) for x in fold_env.split(",") if x}
    ordered = sorted(all_degs, key=lambda d: (d not in fold_degs, d))
    raw = []  # (d, n, s_off, e_off_main, h_eff, fold)
    s_off = 0
    e_off = 0
    for d in ordered:
        n = (counts[d] + N_CORES - 1) // N_CORES
        h = d // 2
        fold = d in fold_degs
        h_eff = h // 2 if fold else h
        raw.append((d, n, s_off, e_off, h_eff, fold))
        s_off += n
        e_off += n * h_eff
    NSLOT = s_off
    E_main = e_off
    fold_src = {}
    e_fold = E_main
    for d, n, so, eo, h_eff, fold in raw:
        if fold:
            fold_src[d] = e_fold
            e_fold += n * h_eff
    E2 = e_fold
    groups = [
        (h_eff, n, so, eo, fold_src.get(d, -1))
        for d, n, so, eo, h_eff, fold in raw
    ]

    in_maps = []
    col_node = np.full((N_CORES, NSLOT), -1, dtype=np.int64)
    for c in range(N_CORES):
        nodes = core_nodes[c]  # global ids, ascending degp
        ndeg = degp[nodes]
        gidx = np.full((2, E2), -1, dtype=np.int64)
        for d, n, so, eo, h_eff, fold in raw:
            nodes_d = nodes[ndeg == d]
            k = len(nodes_d)
            if k == 0:
                continue
            h = d // 2
            col = starts[nodes_d][:, None] + np.arange(d)[None, :]
            valid = np.arange(d)[None, :] < deg[nodes_d][:, None]
            em = np.where(valid, eid_sorted[np.where(valid, col, 0)], -1)
            em = em.reshape(k, 2, h)
            for half in range(2):
                if fold:
                    fo = fold_src[d]
                    gidx[half, eo : eo + k * h_eff] = em[
                        :, half, :h_eff
                    ].ravel()
                    gidx[half, fo : fo + k * h_eff] = em[
                        :, half, h_eff:
                    ].ravel()
                else:
                    gidx[half, eo : eo + k * h] = em[:, half, :].ravel()
            col_node[c, so : so + k] = nodes_d
        X = np.zeros((2, E2, D), dtype=np.float16)
        for half in range(2):
            m = gidx[half] >= 0
            X[half, m] = edge_state16[gidx[half, m]]
        edge_t = np.ascontiguousarray(
            X.transpose(0, 2, 1).reshape(2 * D, E2)
        )  # partitions [0:64)=half0 feats, [64:128)=half1 feats
        nf_t = np.zeros((D, NSLOT), dtype=np.float16)
        vm = col_node[c] >= 0
        nf_t[:, vm] = node_feature[col_node[c][vm]].T
        in_maps.append(
            {"edge_t": edge_t, "nf_t": nf_t, "W": W, "b": b}
        )
    return in_maps, groups, NSLOT, E_main, E2, col_node, N


def _build(groups, NSLOT, E_main, E2, for_sim=False):
    import concourse.bass as bass
    import concourse.mybir as mybir
    import concourse.tile as tile_mod
    from concourse.tile import TileContext

    if for_sim:
        # CoreSim can't digest the walrus single-wait workarounds; build
        # with the stock drain and skip the multi-wait split.
        tile_mod.TileContext._drain_and_barrier = (
            tile_mod.TileContext._orig_drain_and_barrier
        )

    f32 = mybir.dt.float32
    f16 = mybir.dt.float16
    nc = bass.Bass("TRN2", target_bir_lowering=False, debug=False)
    edge_t = nc.declare_dram_parameter("edge_t", [128, E2], f16, isOutput=False)
    nf_t = nc.declare_dram_parameter("nf_t", [64, NSLOT], f16, isOutput=False)
    Wp = nc.declare_dram_parameter("W", [128, D], f16, isOutput=False)
    bp = nc.declare_dram_parameter("b", [64, 1], f32, isOutput=False)
    out_t = nc.declare_dram_parameter("out_t", [64, NSLOT], f16, isOutput=True)

    with TileContext(nc) as tc, nc.allow_low_precision(
        reason="fp16 streams: tolerance is 2e-2; fp16 segment-sum err ~1e-3"
    ):
        with (
            tc.tile_pool(name="const", bufs=1) as cpool,
            tc.tile_pool(name="big", bufs=1) as bigpool,
            tc.tile_pool(name="edges", bufs=EBUF_BUFS) as epool,
            tc.tile_pool(name="scratch", bufs=3) as spool,
            tc.tile_pool(name="psum", bufs=4, space="PSUM") as ppool,
            tc.tile_pool(name="outs", bufs=3) as opool,
        ):
            # Matmul operands must sit at base partition 0 on this HW, so:
            # m1: lhsT=W1 [64,64], rhs=nf [64,:]; m2: lhsT=[W2;W2] [128,64],
            # rhs=agg [128,:] (sums both halves in one K=128 matmul).
            w1 = cpool.tile([64, D], f16)
            nc.scalar.dma_start(out=w1[:], in_=Wp[0:64, :])
            w22 = cpool.tile([128, D], f16)
            nc.scalar.dma_start(out=w22[0:64, :], in_=Wp[64:128, :])
            nc.scalar.dma_start(out=w22[64:128, :], in_=Wp[64:128, :])
            bt = cpool.tile([64, 1], f32)
            nc.scalar.dma_start(out=bt[:], in_=bp[:])

            # Per-slab agg tiles hold an interleaved PAIR of partial sums per
            # node slot (cols 2c/2c+1): the DVE add-tree stops at 2 partials
            # and the PE absorbs the last reduction via two accumulating
            # K=128 matmuls (stride-2 rhs columns). A dense slab depends only
            # on the tree ops that wrote its own tile, so matmul/ACT/out-DMA
            # interleave with the aggregation stream.
            n_slab = (NSLOT + SLAB - 1) // SLAB
            aggs = [
                bigpool.tile([128, 2 * SLAB], f16, name=f"agg{i}", tag=f"agg{i}")
                for i in range(n_slab)
            ]
            def dense_slab(sl):
                s = sl * SLAB
                n = min(SLAB, NSLOT - s)
                nfs = opool.tile([64, SLAB], f16, tag="nfs", name=f"nfs{sl}")
                nc.sync.dma_start(out=nfs[:, :n], in_=nf_t[:, s : s + n])
                ps = ppool.tile(
                    [64, SLAB], f32, space="PSUM", tag="ps", name=f"ps{sl}"
                )
                nc.tensor.matmul(
                    out=ps[:, :n],
                    lhsT=w1[:],
                    rhs=nfs[:, :n],
                    start=True,
                    stop=False,
                )
                pairs = aggs[sl][:, : 2 * n].rearrange(
                    "p (n two) -> p n two", two=2
                )
                nc.tensor.matmul(
                    out=ps[:, :n],
                    lhsT=w22[:],
                    rhs=pairs[:, :, 0],
                    start=False,
                    stop=False,
                )
                nc.tensor.matmul(
                    out=ps[:, :n],
                    lhsT=w22[:],
                    rhs=pairs[:, :, 1],
                    start=False,
                    stop=True,
                )
                ob = opool.tile([64, SLAB], f16, tag="ob", name=f"ob{sl}")
                nc.scalar.activation(
                    out=ob[:, :n],
                    in_=ps[:, :n],
                    func=mybir.ActivationFunctionType.Relu,
                    bias=bt[:],
                )
                nc.scalar.dma_start(out=out_t[:, s : s + n], in_=ob[:, :n])

            # Uniform-size DMA chunks over the contiguous edge stream; the
            # per-group/per-slab reduce segments read from within the chunk.
            # Dense work for a slab is emitted right after the chunk that
            # completes it, so PE/ACT/out-DMA trail the stream closely.
            plan = _chunk_plan(groups, NSLOT, E_main)
            # Interleave fold chunks among plain ones: a fold chunk's
            # accum DMA + tiny tree sit on a latency chain (A-load -> CCE
            # accum -> tree); weaving keeps the DVE fed with plain-tree
            # work while those chains resolve.
            fold_cs = [p for p in plan if p[3]]
            plain_cs = [p for p in plan if not p[3]]
            if fold_cs and plain_cs:
                woven = []
                ratio = len(plain_cs) / len(fold_cs)
                pi = fi = 0
                while pi < len(plain_cs) or fi < len(fold_cs):
                    if pi < len(plain_cs) and (
                        fi >= len(fold_cs) or pi < ratio * (fi + 0.5)
                    ):
                        woven.append(plain_cs[pi])
                        pi += 1
                    else:
                        woven.append(fold_cs[fi])
                        fi += 1
                plan = woven
            last_chunk_of_slab = {}
            for ci, (_, _, segs, _) in enumerate(plan):
                for _, _, _, sl, _ in segs:
                    last_chunk_of_slab[sl] = ci
            edge_qs = [nc.sync, nc.scalar]
            def V(buf, off, stride, cn, k0, k1):
                # [p][cn nodes @ stride][k0:k1] packed-run view
                return buf[:, off : off + cn * stride].rearrange(
                    "p (n k) -> p n k", k=stride
                )[:, :, k0:k1]

            def emit_seg(ebuf, scratch, cur, loff, h, cn, sl, lc):
                # Pairwise fold h (even) down to a 2-partial interleaved pair
                # in aggs[sl][:, 2lc:2lc+2cn]. All adds/copies keep stride-1
                # even-length inner runs at even offsets -> DVE 2x/4x modes.
                buf, off, st, hh = ebuf, loff, h, h
                while hh > 4:
                    k = 2 * (hh // 4)
                    rem = hh - 2 * k  # 0 or 2 (hh even)
                    nh = k + rem
                    nc.vector.tensor_add(
                        V(scratch, cur, nh, cn, 0, k),
                        V(buf, off, st, cn, 0, k),
                        V(buf, off, st, cn, k, 2 * k),
                    )
                    if rem:
                        nc.vector.tensor_copy(
                            V(scratch, cur, nh, cn, k, nh),
                            V(buf, off, st, cn, 2 * k, hh),
                        )
                    buf, off, st, hh = scratch, cur, nh, nh
                    cur += cn * nh
                o2 = aggs[sl][:, 2 * lc : 2 * lc + 2 * cn].rearrange(
                    "p (n two) -> p n two", two=2
                )
                if hh == 4:
                    nc.vector.tensor_add(
                        o2, V(buf, off, st, cn, 0, 2), V(buf, off, st, cn, 2, 4)
                    )
                else:  # hh == 2: pairs already contiguous, straight copy
                    nc.vector.tensor_copy(
                        aggs[sl][:, 2 * lc : 2 * lc + 2 * cn],
                        buf[:, off : off + 2 * cn],
                    )
                return cur

            for ci, (eo, fe, segs, folds) in enumerate(plan):
                ebuf = epool.tile([128, CHUNK_ELEMS], f16, tag="ebuf")
                dma_eng = edge_qs[ci % len(edge_qs)]
                dma_eng.dma_start(out=ebuf[:, :fe], in_=edge_t[:, eo : eo + fe])
                # B half-streams: the DMA's CCE unit adds them onto the A
                # span in SBUF (gpsimd/SWDGE is the only accum-capable path).
                # Accum DMAs above ~0.5MB crash the runtime: split to <=2048
                # elems (128 x 4KB rows) per instruction.
                for dloff, fsrc, flen in folds:
                    for o in range(0, flen, 2048):
                        ln = min(2048, flen - o)
                        nc.gpsimd.dma_start(
                            out=ebuf[:, dloff + o : dloff + o + ln],
                            in_=edge_t[:, fsrc + o : fsrc + o + ln],
                            accum_op=mybir.AluOpType.add,
                        )
                scratch = None
                if any(h > 4 for _, h, _, _, _ in segs):
                    scratch = spool.tile(
                        [128, CHUNK_ELEMS + CHUNK_ELEMS // 4],
                        f16,
                        tag="scr",
                        name=f"scr{ci}",
                    )
                cur = 0
                for loff, h, cn, sl, lc in segs:
                    cur = emit_seg(ebuf, scratch, cur, loff, h, cn, sl, lc)
                for sl in sorted(
                    s for s, lc in last_chunk_of_slab.items() if lc == ci
                ):
                    dense_slab(sl)
    if for_sim:
        # restore the patched drain for subsequent HW builds
        tile_mod.TileContext._drain_and_barrier = (
            tile_mod.TileContext._patched_drain_and_barrier
        )
    else:
        _split_multiwaits(nc)
    return nc


def kernel(node_feature, edge_state, edge_dst, W, b):
    global _last_exec_time_ns, _last_results
    _install_shims()
    from concourse.bass_utils import run_bass_kernel_spmd

    in_maps, groups, NSLOT, E_main, E2, col_node, N = _prepare(
        node_feature, edge_state, edge_dst, W, b
    )
    nc = _build(groups, NSLOT, E_main, E2)
    trace = bool(os.environ.get("GNN_TRACE"))
    res = run_bass_kernel_spmd(
        nc, in_maps, core_ids=list(range(N_CORES)), trace=trace
    )
    _last_exec_time_ns = res.exec_time_ns
    _last_results = res
    out = np.zeros((N, D), dtype=np.float32)
    for c in range(N_CORES):
        ot = np.asarray(res.results[c]["out_t"]).astype(np.float32)
        vm = col_node[c] >= 0
        out[col_node[c][vm]] = ot[:, vm].T
    return out


def last_exec_time_ns():
    return _last_exec_time_ns


def last_results():
    return _last_results

